# revision 5
# baseline (speedup 1.0000x reference)
"""Trainium2 Bass kernel for scrambled-GQA sliding-window attention, v3.

Single SPMD launch across 8 NeuronCores, no collectives:
  per core: QKV projection + RoPE + banded attention for 2 heads (all
  4096 tokens) + output projection restricted to those 2 heads' rows of
  W_proj -> partial (4096, 2048) output; host sums the 8 partials
  (row-split tensor-parallel unshard).

The torch-faithful "scrambled" reshapes in the reference are equivalent to
reinterpreting column slices of qkv = x @ W_attn:
  Q^T_h[d, m*256+t''] = qkv[t''*16+h, c_q(m)*128+d],  c_q(m)=m+2*(m//4), m in [0,16)
  K^T_h'[d, g*1024+u] = qkv[u*4+h', (6g+4)*128+d],    g in [0,4)
  V_h'[g*1024+u, d]   = qkv[u*4+h', (6g+5)*128+d]
Head h attends K/V block h' = h//4 over all rows with band |tq-tk| <= 1023.
RoPE applied to Q,K at position = row index (interleaved pairs).

v3 perf changes vs the two-launch v2 baseline:
  - launch 2 (output proj) folded into launch 1 as dependency-free PE
    filler matmuls interleaved into the attention stream; host reduces
    the per-core partial outputs (fp16 slabs)
  - attention z-interleaved at tk-block granularity: one st psum tile
    (128, 2, 512) per block, exp batched across both heads
  - proj psum->sbuf copies rotated across gpsimd/vector/scalar engines
  - rope saves the Act copy of the swap matmul (DVE reads psum direct)
  - band masks on DVE (fp16 2x mode), denominators per-z via one shared
    psum bank
"""

import math

import numpy as np

B, T, C = 1, 4096, 2048
NH, NKV, HD = 16, 4, 128
WINDOW = 1024
NCORES = 8
P = 128
KC = C // P            # 16 contraction chunks
NM = 16                # scramble chunks (m)
SCALE = 1.0 / math.sqrt(HD)

TQ = 512               # tq tile
NT = T // TQ           # 8 tiles per head
NBLK = T // P          # 32 tk blocks

F16 = np.float16


def _cq(m):
    return m + 2 * (m // 4)


def _block_range(ti):
    b0 = 4 * ti
    return max(0, b0 - 8), min(NBLK - 1, b0 + 11)


def _block_cols(ti, b):
    """Valid tq column range [lo, hi) for block b in tile ti; near-full
    ranges are widened to (0, TQ) (the mask handles the fringe)."""
    D = 512 * ti - 128 * b
    lo = max(0, -1023 - D)
    hi = min(TQ, 1151 - D)
    if lo <= 64:
        lo = 0
    if hi >= TQ - 64:
        hi = TQ
    return D, lo, hi


def _mask_patterns():
    """Partial-band mask tiles keyed by D = tq0 - tkb (multiples of 128)."""
    ds = [640, 768, 896, 1024, -1024, -1152, -1280, -1408]
    tk = np.arange(P)[:, None]
    tq = np.arange(TQ)[None, :]
    masks = {}
    for d in ds:
        masks[d] = (np.abs(d + tq - tk) <= (WINDOW - 1)).astype(np.float32)
    return ds, masks


def host_prep(x, freqs_cis, W_attn, W_proj):
    """Build all per-core / shared numpy inputs (fp16)."""
    x = np.asarray(x, np.float32)
    freqs_cis = np.asarray(freqs_cis, np.float32)
    W_attn = np.asarray(W_attn, np.float32)
    W_proj = np.asarray(W_proj, np.float32)

    xT = np.ascontiguousarray(x[0].T)            # (C, T) = (2048, 4096)

    # RoPE tables, (128, T): rows 2i,2i+1 = cos(ang[:, i]); sin signed.
    cos = np.repeat(freqs_cis[:, :, 0].T, 2, axis=0)                       # (128, T)
    sin_base = freqs_cis[:, :, 1].T                                        # (64, T)
    sin = np.empty((P, T), np.float32)
    sin[0::2] = -sin_base
    sin[1::2] = sin_base
    cos = cos.astype(F16)
    sin = sin.astype(F16)

    # pair-swap matrix (symmetric): row 2i <-> row 2i+1
    mt = np.zeros((P, P), np.float32)
    ii = np.arange(0, P, 2)
    mt[ii, ii + 1] = 1.0
    mt[ii + 1, ii] = 1.0
    mt = mt.astype(F16)

    mask_ds, masks = _mask_patterns()
    # additive form: 0 in-band, -60000 out-of-band (exp underflows to 0)
    masks_arr = np.ascontiguousarray(
        np.stack([-60000.0 * (1.0 - masks[d]) for d in mask_ds],
                 axis=1)).astype(F16)                               # (128, 8, 512)
    eye = np.eye(P, dtype=F16)

    wa3 = W_attn.reshape(KC, P, 24, HD)          # [kc][p][blk][d]
    wq = np.stack(
        [np.ascontiguousarray(
            wa3[:, :, _cq(m), :].transpose(1, 0, 2).reshape(P, KC * HD))
         for m in range(NM)]
    ).astype(F16)                                # (16, 128, 2048)
    wk = np.stack(
        [np.ascontiguousarray(
            wa3[:, :, 6 * g + 4, :].transpose(1, 0, 2).reshape(P, KC * HD))
         for g in range(NKV)]
    ).astype(F16)                                # (4, 128, 2048)
    wv = np.ascontiguousarray(
        np.concatenate([wa3[:, :, 6 * g + 5, :] for g in range(NKV)], axis=2)
        .transpose(1, 0, 2)
        .reshape(P, KC, NKV * HD)
    ).astype(F16)                                # (128, 16, 512)

    wp_all = np.ascontiguousarray(W_proj.reshape(NH, HD, C)).astype(F16)  # (16,128,2048)

    per_core = []
    for c in range(NCORES):
        hp = c // 2
        cols = np.concatenate([np.arange(256) * 16 + (2 * c + z) for z in (0, 1)])
        xq = np.ascontiguousarray(
            xT[:, cols].reshape(KC, P, 512).transpose(1, 0, 2)).astype(F16)
        ucols = np.arange(1024) * 4 + hp
        xkv = np.ascontiguousarray(
            xT[:, ucols].reshape(KC, P, 1024).transpose(1, 0, 2)).astype(F16)
        wp = np.ascontiguousarray(wp_all[2 * c:2 * c + 2])   # (2, 128, 2048)
        per_core.append(
            dict(xq=xq, xkv=xkv, wq=wq, wk=wk, wv=wv, cos=cos, sin=sin,
                 mt=mt, ones=np.ones((P, P), F16), masks=masks_arr, wp=wp,
                 eye=eye)
        )
    return per_core, mask_ds


# ---------------------------------------------------------------------------
# numpy emulation of the exact device algorithm (validates all index math)
# ---------------------------------------------------------------------------

def emulate(x, freqs_cis, W_attn, W_proj):
    per_core, mask_ds = host_prep(x, freqs_cis, W_attn, W_proj)
    _, masks = _mask_patterns()
    pouts = []
    for c in range(NCORES):
        d = per_core[c]
        xq = d["xq"].astype(np.float32).transpose(1, 0, 2).reshape(C, 512)
        xkv = d["xkv"].astype(np.float32).transpose(1, 0, 2).reshape(C, 1024)
        cos = d["cos"].astype(np.float32)
        sin = d["sin"].astype(np.float32)
        mt = d["mt"].astype(np.float32)
        qr = np.zeros((2, P, T), np.float32)
        for m in range(NM):
            wq_full = (d["wq"][m].astype(np.float32)
                       .reshape(P, KC, HD).transpose(1, 0, 2).reshape(C, HD))
            qt = wq_full.T @ xq                               # (128, 512) [d,(z,t'')]
            qt = qt.astype(F16).astype(np.float32)
            qsw = mt @ qt
            c2 = np.concatenate([cos[:, m * 256:(m + 1) * 256]] * 2, axis=1)
            s2 = np.concatenate([sin[:, m * 256:(m + 1) * 256]] * 2, axis=1)
            qt = (qt * c2 + qsw * s2).astype(F16).astype(np.float32)
            qr[0, :, m * 256:(m + 1) * 256] = qt[:, :256]
            qr[1, :, m * 256:(m + 1) * 256] = qt[:, 256:]
        kr = np.zeros((P, T), np.float32)
        for g in range(NKV):
            wkg = (d["wk"][g].astype(np.float32)
                   .reshape(P, KC, HD).transpose(1, 0, 2).reshape(C, HD))
            kt = (wkg.T @ xkv).astype(F16).astype(np.float32)
            ksw = mt @ kt
            sl = slice(g * 1024, (g + 1) * 1024)
            kr[:, sl] = (kt * cos[:, sl] + ksw * sin[:, sl]).astype(F16)
        vall = np.zeros((P, 8, 512), np.float32)
        wv_full = d["wv"].astype(np.float32).transpose(1, 0, 2).reshape(C, 512)
        for ut in range(8):
            vall[:, ut, :] = (xkv[:, ut * 128:(ut + 1) * 128].T
                              @ wv_full).astype(F16)
        yz = np.zeros((2, P, T), np.float32)      # per-head y^T (fp16-rounded)
        for ti in range(NT):
            blo, bhi = _block_range(ti)
            for z in range(2):
                q_tile = qr[z, :, ti * TQ:(ti + 1) * TQ]
                y_acc = np.zeros((P, TQ), np.float32)
                s_acc = np.zeros((TQ,), np.float32)
                for b in range(blo, bhi + 1):
                    D, lo, hi = _block_cols(ti, b)
                    st = kr[:, b * P:(b + 1) * P].T @ q_tile[:, lo:hi]
                    pt = np.exp(SCALE * st).astype(F16).astype(np.float32)
                    if D in masks:
                        pt = pt * masks[D][:, lo:hi]
                    g, ub = b // 8, b % 8
                    vblk = vall[:, ub, g * HD:(g + 1) * HD]   # (128u, 128d)
                    y_acc[:, lo:hi] += vblk.T @ pt
                    s_acc[lo:hi] += pt.sum(axis=0)
                yz[z, :, ti * TQ:(ti + 1) * TQ] = (
                    y_acc / s_acc[None, :]).astype(F16)
        # output projection for this core's 2 heads over all tokens
        wp = d["wp"].astype(np.float32)           # (2, 128, 2048)
        pout = np.zeros((T, C), np.float32)
        for z in range(2):
            pout += yz[z].T @ wp[z]
        pouts.append(pout.astype(F16))
    out = np.zeros((T, C), np.float32)
    for p in pouts:
        out += p.astype(np.float32)
    return out.reshape(B, T, C)


# ---------------------------------------------------------------------------
# Bass program
# ---------------------------------------------------------------------------

def build_launch1():
    import concourse.bacc as bacc
    import concourse.mybir as mybir
    import concourse.tile as tile

    f32 = mybir.dt.float32
    f16 = mybir.dt.float16
    MUL = mybir.AluOpType.mult
    ADD = mybir.AluOpType.add

    nc = bacc.Bacc("TRN2", target_bir_lowering=False, debug=False)

    xq_d = nc.dram_tensor("xq", (P, KC, 512), f16, kind="ExternalInput")
    xkv_d = nc.dram_tensor("xkv", (P, KC, 1024), f16, kind="ExternalInput")
    wq_d = nc.dram_tensor("wq", (NM, P, KC * HD), f16, kind="ExternalInput")
    wk_d = nc.dram_tensor("wk", (NKV, P, KC * HD), f16, kind="ExternalInput")
    wv_d = nc.dram_tensor("wv", (P, KC, 512), f16, kind="ExternalInput")
    cos_d = nc.dram_tensor("cos", (P, T), f16, kind="ExternalInput")
    sin_d = nc.dram_tensor("sin", (P, T), f16, kind="ExternalInput")
    mt_d = nc.dram_tensor("mt", (P, P), f16, kind="ExternalInput")
    ones_d = nc.dram_tensor("ones", (P, P), f16, kind="ExternalInput")
    masks_d = nc.dram_tensor("masks", (P, 8, TQ), f16, kind="ExternalInput")
    eye_d = nc.dram_tensor("eye", (P, P), f16, kind="ExternalInput")
    wp_d = nc.dram_tensor("wp", (2, P, C), f16, kind="ExternalInput")
    pout_d = nc.dram_tensor("pout", (T, C), f16, kind="ExternalOutput")

    mask_ds, _ = _mask_patterns()
    mask_idx = {d: i for i, d in enumerate(mask_ds)}

    with tile.TileContext(nc) as tc:
        with tc.tile_pool(name="persist", bufs=1) as persist:
            qr_t = [persist.tile([P, 2, TQ], f16, tag=f"qr{t}", name=f"qr{t}")
                    for t in range(NT)]
            kr = persist.tile([P, T], f16, tag="kr", name="kr")
            vall = persist.tile([P, 8, 512], f16, tag="vall", name="vall")
            ones = persist.tile([P, P], f16, tag="ones", name="ones")
            mt_s = persist.tile([P, P], f16, tag="mt", name="mt_s")
            masks_s = persist.tile([P, 8, TQ], f16, tag="masks", name="masks_s")
            wp_s = persist.tile([P, 2, C], f16, tag="wp", name="wp_s")
            eye_s = persist.tile([P, P], f16, tag="eye", name="eye_s")


            def rope(wpool, swp, sb, tsl, n, out_ap, view, vtab):
                """out = sb*cos + (M @ sb)*sin; sb is flat (128, 512) f16 SBUF.
                cos/sin slices [tsl] of width n are streamed from DRAM."""
                ctab = wpool.tile([P, TQ], f16, tag="ctab", bufs=2, name="ctab")
                nc.sync.dma_start(ctab[:, :n], cos_d.ap()[:, tsl])
                stab = wpool.tile([P, TQ], f16, tag="stab", bufs=2, name="stab")
                nc.sync.dma_start(stab[:, :n], sin_d.ap()[:, tsl])
                sw_t = swp.tile([P, 2, TQ], f32, tag="st", bufs=2,
                                name="sw_t")
                sw_ps = sw_t[:, 0]
                nc.tensor.matmul(sw_ps, mt_s[:], sb, start=True, stop=True)
                t1 = wpool.tile([P, TQ], f16, tag="rope_t1", bufs=1, name="rope_t1")
                nc.vector.tensor_tensor(view(t1), view(sb), vtab(ctab), MUL)
                t2 = wpool.tile([P, TQ], f16, tag="rope_t2", bufs=1, name="rope_t2")
                nc.vector.tensor_tensor(view(t2), view(sw_ps), vtab(stab), MUL)
                nc.vector.tensor_tensor(out_ap, view(t1), view(t2), ADD)

            with tc.tile_pool(name="wstream", bufs=2) as wsp, \
                 tc.tile_pool(name="wkpool", bufs=4) as wkp, \
                 tc.tile_pool(name="qp", bufs=1) as qp, \
                 tc.tile_pool(name="xkvp", bufs=1) as xkvp, \
                 tc.tile_pool(name="asb", bufs=2) as asb, \
                 tc.tile_pool(name="ptp", bufs=6) as ptp, \
                 tc.tile_pool(name="pap", bufs=2) as pap, \
                 tc.tile_pool(name="prj", bufs=6) as prj:
                xq_s = qp.tile([P, KC, 512], f16, tag="xq", bufs=1,
                               name="xq_s")
                xkv_s = xkvp.tile([P, KC, 1024], f16, tag="xkv", name="xkv_s")

                # ---- V (kc-outer: PE starts after first ~0.4MB of DMA) ----
                wk_ss = []
                with tc.tile_pool(name="vp", bufs=4) as vp, \
                     tc.tile_pool(name="vps", bufs=8, space="PSUM") as vps:
                    vpss = [vps.tile([P, TQ], f32, tag="vpsum",
                                     name="vpsum") for _ in range(8)]
                    for kc in range(KC):
                        nc.sync.dma_start(xkv_s[:, kc], xkv_d.ap()[:, kc])
                        wv_c = vp.tile([P, 1, 512], f16, tag="wv",
                                       bufs=3, name="wv_c")
                        if kc == 0:
                            nc.scalar.dma_start(wv_c[:, 0], wv_d.ap()[:, kc])
                        else:
                            nc.sync.dma_start(wv_c[:, 0], wv_d.ap()[:, kc])
                        if kc == 0:
                            nc.sync.dma_start(mt_s[:], mt_d.ap())
                            nc.sync.dma_start(ones[:], ones_d.ap())
                            nc.sync.dma_start(eye_s[:], eye_d.ap())
                        for ut in range(8):
                            nc.tensor.matmul(
                                vpss[ut],
                                xkv_s[:, kc, ut * P:(ut + 1) * P],
                                wv_c[:, 0],
                                start=(kc == 0), stop=(kc == KC - 1))
                        if kc >= 12:
                            # wk prefetch at the tail of the V DMA stream
                            g = kc - 12
                            wk_s = wkp.tile([P, KC, HD], f16, tag="wk",
                                            bufs=4, name="wk_s")
                            nc.sync.dma_start(
                                wk_s[:],
                                wk_d.ap()[g].rearrange(
                                    "p (kc d) -> p kc d", d=HD))
                            wk_ss.append(wk_s)
                    for ut in range(8):
                        if ut % 2 == 0:
                            nc.scalar.copy(vall[:, ut], vpss[ut])
                        else:
                            nc.vector.tensor_copy(vall[:, ut], vpss[ut])

                # PSUM pools for the merged stream: exactly 8 banks:
                # qkv/proj 2 + rope-sw 1 + st 2 + y 2 + sums 1.
                pp = tc.alloc_tile_pool(name="pp", bufs=2, space="PSUM")
                stp = tc.alloc_tile_pool(name="stp", bufs=2, space="PSUM")
                yp = tc.alloc_tile_pool(name="yp", bufs=2, space="PSUM")

                # ---- K (with masks/wp/xq prefetch spread through it) ----
                # Lag the rope of chain i behind the matmuls of chain i+1 so
                # the rope's swap-matmul never stalls PE on the Act copy.
                pend_rope = None
                for g in range(NKV):
                    for ut in range(2):
                        ps = pp.tile([P, TQ], f32, tag="qkv",
                                     name="kpsum")
                        for kc in range(KC):
                            nc.tensor.matmul(
                                ps, wk_ss[g][:, kc],
                                xkv_s[:, kc, ut * TQ:(ut + 1) * TQ],
                                start=(kc == 0), stop=(kc == KC - 1))
                        ksb = wsp.tile([P, TQ], f16, tag="sbr", bufs=2,
                                       name="ksb")
                        nc.scalar.copy(ksb, ps)
                        if pend_rope is not None:
                            pend_rope()
                        sl = slice(g * 1024 + ut * TQ,
                                   g * 1024 + (ut + 1) * TQ)
                        pend_rope = (lambda ksb=ksb, sl=sl: rope(
                            wsp, stp, ksb, sl, TQ, kr[:, sl],
                            lambda a: a, lambda tb: tb[:, :TQ]))
                        ku = 2 * g + ut
                        if ku == 2:
                            nc.sync.dma_start(masks_s[:], masks_d.ap())
                        elif ku == 3:
                            nc.sync.dma_start(wp_s[:, 0], wp_d.ap()[0])
                            nc.sync.dma_start(wp_s[:, 1], wp_d.ap()[1])
                        elif ku >= 6:
                            for kq in range(8 * (ku - 6), 8 * (ku - 5)):
                                nc.sync.dma_start(xq_s[:, kq],
                                                  xq_d.ap()[:, kq])
                pend_rope()

                # wq prefetch pipeline: tile for m is DMA'd two qprojs (one
                # full attention tile) ahead of its use.
                wq_tiles = {}

                def wq_fetch(m):
                    wq_s = wsp.tile([P, KC, HD], f16, tag="w", bufs=3,
                                    name="wq_s")
                    nc.sync.dma_start(
                        wq_s[:],
                        wq_d.ap()[m].rearrange("p (kc d) -> p kc d", d=HD))
                    wq_tiles[m] = wq_s

                def qproj_chain(m):
                    ps = pp.tile([P, TQ], f32, tag="qkv", name="qpsum")
                    wq_s = wq_tiles.pop(m)
                    for kc in range(KC):
                        nc.tensor.matmul(
                            ps, wq_s[:, kc], xq_s[:, kc],
                            start=(kc == 0), stop=(kc == KC - 1))
                    qsb = wsp.tile([P, TQ], f16, tag="sbr", bufs=2, name="qsb")
                    nc.scalar.copy(qsb, ps)
                    return qsb

                def qproj_rope(m, qsb):
                    ti, half = m // 2, m % 2
                    rope(wsp, stp, qsb,
                         slice(m * 256, (m + 1) * 256), 256,
                         qr_t[ti][:, :, half * 256:half * 256 + 256],
                         lambda a: a.rearrange("p (z t) -> p z t", z=2),
                         lambda tb: tb[:, None, :256].broadcast_to(
                             (P, 2, 256)))

                # ---- output-projection chunk queue (PE filler work) ----
                proj_queue = []
                copy_rotation = ["scalar", "vector"]
                copy_n = [0]

                def emit_proj_chunk(eng=None, pool=None):
                    if not proj_queue:
                        return
                    ti, tt, ct = proj_queue.pop(0)
                    if pool is None:
                        pool = pp
                    ps = pool.tile([P, TQ], f32, tag="qkv", name="prpsum")
                    for z in range(2):
                        nc.tensor.matmul(
                            ps,
                            qr_t[ti][:, z, tt * P:(tt + 1) * P],
                            wp_s[:, z, ct * TQ:(ct + 1) * TQ],
                            start=(z == 0), stop=(z == 1))
                    o_sb = prj.tile([P, TQ], f16, tag="osb", bufs=6,
                                    name="o_sb")
                    if eng is None:
                        eng = copy_rotation[copy_n[0] % len(copy_rotation)]
                        copy_n[0] += 1
                    if eng == "vector":
                        nc.vector.tensor_copy(o_sb, ps)
                    else:
                        nc.scalar.copy(o_sb, ps)
                    nc.sync.dma_start(
                        pout_d.ap()[ti * TQ + tt * P: ti * TQ + (tt + 1) * P,
                                    ct * TQ:(ct + 1) * TQ],
                        o_sb)

                def queue_proj(ti):
                    for tt in range(4):
                        for ct in range(4):
                            proj_queue.append((ti, tt, ct))

                def attention(ti):
                    """z-interleaved banded attention for q-tile ti."""
                    blo, bhi = _block_range(ti)
                    binfo = {b: _block_cols(ti, b)
                             for b in range(blo, bhi + 1)}

                    def isfull(b):
                        return binfo[b][1] == 0 and binfo[b][2] == TQ

                    clean = [b for b in sorted(binfo)
                             if isfull(b) and binfo[b][0] not in mask_idx]
                    mfull = [b for b in sorted(binfo)
                             if isfull(b) and binfo[b][0] in mask_idx]
                    part = [b for b in sorted(binfo) if not isfull(b)]
                    order = clean + mfull + part
                    y_ps = [yp.tile([P, TQ], f32, tag="y", name=f"y{z}")
                            for z in range(2)]
                    pacc = pap.tile([P, 2, TQ], f16, tag="pacc", name="pacc")
                    emit_proj_chunk()
                    emit_proj_chunk()
                    for bi, b in enumerate(order):
                        D, lo, hi = binfo[b]
                        full = (lo == 0 and hi == TQ)
                        st_ps = stp.tile([P, 2, TQ], f32, tag="st",
                                         name="st_ps")
                        masked = D in mask_idx
                        for z in range(2):
                            nc.tensor.matmul(
                                st_ps[:, z, lo:hi],
                                kr[:, b * P:(b + 1) * P],
                                qr_t[ti][:, z, lo:hi],
                                start=True, stop=not masked)
                            if masked:
                                # band mask as additive bias on PE: the
                                # out-of-band -60000 rows underflow exp to 0
                                nc.tensor.matmul(
                                    st_ps[:, z, lo:hi], eye_s[:],
                                    masks_s[:, mask_idx[D], lo:hi],
                                    start=False, stop=True)
                        pt = ptp.tile([P, 2, TQ], f16, tag="pt", name="pt")
                        if full:
                            nc.scalar.activation(
                                pt, st_ps,
                                mybir.ActivationFunctionType.Exp,
                                scale=SCALE)
                        else:
                            nc.scalar.activation(
                                pt[:, :, lo:hi], st_ps[:, :, lo:hi],
                                mybir.ActivationFunctionType.Exp,
                                scale=SCALE)
                        # denominator accumulation on DVE (wide f16 ops)
                        if bi == 0:
                            nc.vector.tensor_copy(pacc, pt)
                        elif full:
                            nc.vector.tensor_tensor(pacc, pacc, pt, ADD)
                        else:
                            for z in range(2):
                                nc.vector.tensor_tensor(
                                    pacc[:, z, lo:hi], pacc[:, z, lo:hi],
                                    pt[:, z, lo:hi], ADD)
                        # filler BETWEEN st and y so PE never idles while
                        # the Act engine computes exp for this block
                        emit_proj_chunk()
                        for z in range(2):
                            nc.tensor.matmul(
                                y_ps[z][:, lo:hi],
                                vall[:, b % 8, (b // 8) * HD:
                                     (b // 8 + 1) * HD],
                                pt[:, z, lo:hi],
                                start=(bi == 0), stop=(bi == len(order) - 1))

                    def fin1():
                        # per-z denominator -> reciprocal row (fp16)
                        r_sbs = []
                        for z in range(2):
                            s_pt = stp.tile([P, 2, TQ], f32, tag="st",
                                            bufs=2, name="s_pt")
                            s_ps = s_pt[0:1, 0]
                            nc.tensor.matmul(s_ps, ones[:, 0:1], pacc[:, z],
                                             start=True, stop=True)
                            r_sb = asb.tile([1, TQ], f32, tag="rsb",
                                            name="r_sb")
                            nc.vector.reciprocal_approx_fast(r_sb, s_ps)
                            r_sbs.append(r_sb)
                        return r_sbs

                    def fin2(r_sbs, ti=ti, y_ps=y_ps):
                        # broadcast + normalize, deferred one qproj chain so
                        # the gpsimd/DVE chain never gates the next tile's
                        # proj-chunk fillers
                        for z in range(2):
                            rb_sb = asb.tile([P, TQ], f32, tag="rbsb",
                                             name="rb_sb")
                            nc.gpsimd.partition_broadcast(rb_sb, r_sbs[z])
                            nc.vector.tensor_tensor(
                                qr_t[ti][:, z], y_ps[z], rb_sb, MUL)
                    return fin1, fin2

                # ---- merged Q-proj + attention + out-proj stream ----
                tile_order = list(range(NT))
                mseq = [m for ti in tile_order for m in (2 * ti, 2 * ti + 1)]
                wq_fetch(mseq[0])
                wq_fetch(mseq[1])
                pend_fin2 = None
                for i, ti in enumerate(tile_order):
                    m0, m1 = 2 * ti, 2 * ti + 1
                    qsb0 = qproj_chain(m0)
                    if i * 2 + 2 < len(mseq):
                        wq_fetch(mseq[i * 2 + 2])
                    if pend_fin2 is not None:
                        # prev tile's broadcast+normalize lands here, after a
                        # full qproj chain has covered the recip latency; the
                        # prev tile's proj chunks (emitted below) need it
                        pend_fin2()
                        pend_fin2 = None
                    qsb1 = qproj_chain(m1)
                    if i * 2 + 3 < len(mseq):
                        wq_fetch(mseq[i * 2 + 3])
                    emit_proj_chunk(eng="scalar")
                    qproj_rope(m0, qsb0)
                    emit_proj_chunk(eng="scalar")
                    qproj_rope(m1, qsb1)
                    emit_proj_chunk(eng="scalar")
                    emit_proj_chunk(eng="scalar")
                    fin1, fin2 = attention(ti)
                    rs = fin1()
                    pend_fin2 = (lambda rs=rs, fin2=fin2: fin2(rs))
                    queue_proj(ti)
                pend_fin2()
                # final drain: attention psum pools are done — recycle their
                # banks into a deeper proj pool so the tail pipelines at PE
                # rate instead of copy-latency rate
                yp.release()
                stp.release()
                dp = tc.alloc_tile_pool(name="dp", bufs=4, space="PSUM")
                while proj_queue:
                    emit_proj_chunk(pool=dp)
                dp.release()
                pp.release()

    nc.compile()
    return nc


_cache = {}


def kernel(x, freqs_cis, W_attn, W_proj, _trace=False, _timing=None):
    from concourse.bass_utils import run_bass_kernel_spmd

    per_core, _ = host_prep(x, freqs_cis, W_attn, W_proj)

    if "l1" not in _cache:
        _cache["l1"] = build_launch1()

    kw = dict(trace=True, trace_cores=list(range(NCORES))) if _trace else {}
    res1 = run_bass_kernel_spmd(_cache["l1"], per_core, list(range(NCORES)), **kw)
    out = np.zeros((T, C), np.float32)
    for c in range(NCORES):
        out += res1.results[c]["pout"].astype(np.float32)

    if _timing is not None:
        _timing["l1_ns"] = res1.exec_time_ns
        _timing["res1"] = res1
    return out.reshape(B, T, C)
